# revision 17
# baseline (speedup 1.0000x reference)
"""CoarseAlignment kernel for 8 Trainium2 NeuronCores.

Device (SPMD, 8 cores): similarity matrix at fp32 accuracy via a bf16
hi/lo x3 matmul decomposition (hi@hi + hi@lo + lo@hi, 3 cyc/row vs
native fp32's 4), post-scaled by on-device feature reciprocal norms;
row argmax per src shard and col argmax via PE transposes of the
resident scaled tiles (no second matmul pass).  The fp32-level
accuracy is required: the min top-2 argmax gap is ~9e-7 and argmax
flips cascade through RANSAC into a completely different output.
Host: gather/unshard + the tiny data-dependent tail (RANSAC DLT/eigh
over 10000 hypotheses of which ~11 are sample_ok, polish, 480x640
warp) on jax's CPU backend so its near-degenerate final eigenvector
(lambda_min/lambda_2 = 0.287/0.377) and in-image projective pole match
the reference's arithmetic exactly.
"""
import os
import sys
import numpy as np

for _p in ("/opt/trn_rl_repo", "/root/.axon_site/_ro/trn_rl_repo"):
    if os.path.isdir(_p) and _p not in sys.path:
        sys.path.insert(0, _p)

import concourse.bass as bass
import concourse.mybir as mybir
from concourse import bacc
from concourse.tile import TileContext
from concourse.bass_utils import run_bass_kernel_spmd

N_SRC, N_DST, D = 8192, 2048, 1024
H_IMG, W_IMG = 480, 640
MAX_ITER = 10000
INL_TH = np.float32(0.05)
EPS = np.float32(1e-8)
CHUNK = 100
NCORES = 8
MSH = N_SRC // NCORES            # 1024 src rows per core
NKT = D // 128                   # 8 contraction tiles
PKW = MSH + N_DST                # 3072 packed columns per k-tile

_cached = {}


def _build_phase1():
    nc = bacc.Bacc(num_devices=NCORES)
    pk = nc.declare_dram_parameter("pk", [NKT, 128, 2 * PKW],
                                   mybir.dt.bfloat16, isOutput=False)
    nn12_o = nc.declare_dram_parameter("nn12", [MSH // 128, 128],
                                       mybir.dt.uint32, isOutput=True)
    colval_o = nc.declare_dram_parameter("colval", [N_DST // 128, 128],
                                         mybir.dt.float32, isOutput=True)
    colidx_o = nc.declare_dram_parameter("colidx", [N_DST // 128, 128],
                                         mybir.dt.uint32, isOutput=True)
    f32 = mybir.dt.float32
    NMT, NDT = MSH // 128, N_DST // 128
    from concourse.masks import make_identity
    with TileContext(nc) as tc:
        with tc.tile_pool(name="pks", bufs=NKT) as ppk, \
             tc.tile_pool(name="cst", bufs=1) as cst:
            P = []
            for kt in range(NKT):
                t = ppk.tile([128, 2 * PKW], mybir.dt.bfloat16, tag="pk",
                             name=f"p{kt}")
                nc.sync.dma_start(t[:], pk.ap()[kt])
                P.append(t)

            ones = cst.tile([128, 1], f32, name="ones")
            nc.gpsimd.memset(ones[:], 1.0)
            eps_t = cst.tile([1, 1], f32, name="eps_t")
            nc.gpsimd.memset(eps_t[:], float(EPS))
            ident = cst.tile([128, 128], f32, name="ident")
            make_identity(nc, ident)
            bc_dst = cst.tile([128, N_DST], f32, name="bc_dst")
            rst = cst.tile([128, MSH // 128], f32, name="rst")

            # reciprocal norms via ACT Square + ones-matmul (sum over the
            # contraction partition dim), one PSUM accumulation group
            with tc.tile_pool(name="sqp", bufs=4) as sqp, \
                 tc.tile_pool(name="nrm", bufs=1) as nrmp, \
                 tc.tile_pool(name="psn", bufs=1, space="PSUM") as psn:
                pnorm = psn.tile([1, PKW], f32, name="pnorm")
                for kt in range(NKT):
                    for j in range(PKW // 1536):
                        s = sqp.tile([128, 1536], f32, tag="s", name="s")
                        nc.vector.tensor_add(
                            s[:], P[kt][:, j * 1536:(j + 1) * 1536],
                            P[kt][:, PKW + j * 1536:PKW + (j + 1) * 1536])
                        sq = sqp.tile([128, 1536], f32, tag="sq", name="sq")
                        nc.scalar.activation(
                            sq[:], s[:], mybir.ActivationFunctionType.Square)
                        for i in range(3):
                            c0 = j * 1536 + i * 512
                            nc.tensor.matmul(
                                pnorm[:, c0:c0 + 512], ones[:],
                                sq[:, i * 512:(i + 1) * 512],
                                start=(kt == 0), stop=(kt == NKT - 1))
                rec = nrmp.tile([1, PKW], f32, name="rec")
                nc.scalar.sqrt(rec[:], pnorm[:])
                nc.vector.tensor_scalar_add(rec[:], rec[:], eps_t[:])
                nc.vector.reciprocal(rec[:], rec[:])
                nc.gpsimd.partition_broadcast(bc_dst[:], rec[:, MSH:PKW])
                rs_dram = nc.dram_tensor("rs_dram", [MSH], f32)
                nc.sync.dma_start(rs_dram.ap(), rec[:, 0:MSH])
                nc.sync.dma_start(
                    rst[:], rs_dram.ap().rearrange("(mt p) -> p mt", p=128))

            # row pass: fp32 sim, scaled by 1/dst-norm; all 8 scaled tiles
            # stay resident for the transpose-based col argmax
            with tc.tile_pool(name="psr", bufs=3, space="PSUM") as psr, \
                 tc.tile_pool(name="pst", bufs=2, space="PSUM") as pst, \
                 tc.tile_pool(name="scl", bufs=2) as sclp, \
                 tc.tile_pool(name="simt", bufs=1) as simtp, \
                 tc.tile_pool(name="mx", bufs=2) as mxp:
                simT = [simtp.tile([128, MSH], f32, tag=f"sT{d}",
                                   name=f"sT{d}") for d in range(NDT)]
                scaled_tiles = []
                for mt in range(NMT):
                    scaled = sclp.tile([128, N_DST], f32, tag="scaled",
                                       name="scaled")
                    scaled_tiles.append(scaled)
                    for half in range(2):
                        ps = psr.tile([128, 1024], f32, tag="psr", name="ps")
                        for nt in range(2):
                            col0 = half * 1024 + nt * 512
                            for kt in range(NKT):
                                hm = P[kt][:, mt * 128:(mt + 1) * 128]
                                lm = P[kt][:, PKW + mt * 128:
                                           PKW + (mt + 1) * 128]
                                hd = P[kt][:, MSH + col0:MSH + col0 + 512]
                                ld = P[kt][:, PKW + MSH + col0:
                                           PKW + MSH + col0 + 512]
                                nc.tensor.matmul(
                                    ps[:, nt * 512:(nt + 1) * 512], hm, hd,
                                    start=(kt == 0), stop=False)
                                nc.tensor.matmul(
                                    ps[:, nt * 512:(nt + 1) * 512], hm, ld,
                                    start=False, stop=False)
                                nc.tensor.matmul(
                                    ps[:, nt * 512:(nt + 1) * 512], lm, hd,
                                    start=False, stop=(kt == NKT - 1))
                        nc.vector.tensor_mul(
                            scaled[:, half * 1024:(half + 1) * 1024],
                            ps[:], bc_dst[:, half * 1024:(half + 1) * 1024])
                    nc.vector.tensor_scalar_mul(scaled[:], scaled[:],
                                                rst[:, mt:mt + 1])
                    mx = mxp.tile([128, 8], f32, tag="mx", name="mx")
                    idx = mxp.tile([128, 8], mybir.dt.uint32, tag="idx",
                                   name="idx")
                    nc.vector.max(mx[:], scaled[:])
                    nc.vector.max_index(idx[:], mx[:], scaled[:])
                    nc.sync.dma_start(nn12_o.ap()[mt], idx[:, 0:1])
                    for dt in range(NDT):
                        pt = pst.tile([128, 128], f32, tag="pt", name="pt")
                        nc.tensor.transpose(
                            pt[:], scaled[:, dt * 128:(dt + 1) * 128],
                            ident[:])
                        nc.scalar.copy(
                            simT[dt][:, mt * 128:(mt + 1) * 128], pt[:])

                # col argmax tail: only the DVE max chains remain after the
                # last row tile's transposes
                with tc.tile_pool(name="mxc", bufs=2) as mxcp:
                    for dt in range(NDT):
                        mx = mxcp.tile([128, 8], f32, tag="mxc", name="mxc")
                        idx = mxcp.tile([128, 8], mybir.dt.uint32, tag="idxc",
                                        name="idxc")
                        nc.vector.max(mx[:], simT[dt][:])
                        nc.vector.max_index(idx[:], mx[:], simT[dt][:])
                        nc.sync.dma_start(colval_o.ap()[dt], mx[:, 0:1])
                        nc.sync.dma_start(colidx_o.ap()[dt], idx[:, 0:1])
    nc.compile()
    return nc


def _run_phase1(feature_src, feature_dst):
    if "nc1" not in _cached:
        _cached["nc1"] = _build_phase1()
    nc = _cached["nc1"]
    import ml_dtypes
    bf16 = ml_dtypes.bfloat16
    srcT = np.ascontiguousarray(feature_src.T)          # [D, N_SRC]
    dstT = np.ascontiguousarray(feature_dst.T)          # [D, N_DST]
    in_maps = []
    dst_hi = dstT.astype(bf16)
    dst_lo = (dstT - dst_hi.astype(np.float32)).astype(bf16)
    for c in range(NCORES):
        blk = srcT[:, c * MSH:(c + 1) * MSH]
        src_hi = blk.astype(bf16)
        src_lo = (blk - src_hi.astype(np.float32)).astype(bf16)
        hi = np.concatenate([src_hi.reshape(NKT, 128, MSH),
                             dst_hi.reshape(NKT, 128, N_DST)], axis=2)
        lo = np.concatenate([src_lo.reshape(NKT, 128, MSH),
                             dst_lo.reshape(NKT, 128, N_DST)], axis=2)
        pk = np.concatenate([hi, lo], axis=2)           # [NKT,128,2*PKW] bf16
        in_maps.append({"pk": np.ascontiguousarray(pk)})
    res = run_bass_kernel_spmd(nc, in_maps, list(range(NCORES)))
    nn12 = np.concatenate([res.results[c]["nn12"].reshape(-1)
                           for c in range(NCORES)]).astype(np.int64)
    vals = np.stack([res.results[c]["colval"].reshape(-1)
                     for c in range(NCORES)])            # [8, N_DST]
    idxs = np.stack([res.results[c]["colidx"].reshape(-1)
                     for c in range(NCORES)])            # [8, N_DST]
    cbest = np.argmax(vals, axis=0)                      # first-max tiebreak
    nn21 = (idxs[cbest, np.arange(N_DST)] + cbest * MSH).astype(np.int64)
    return nn12, nn21


def _tail_np(I_src, h_src, w_src, h_dst, w_dst, sample_idx, nn12, nn21):
    # numpy fallback (used only if no jax CPU backend is importable)
    mutual = nn21[nn12] == np.arange(N_SRC)
    kp_src = np.stack([h_src, w_src], -1)
    kp_dst = np.stack([h_dst[nn12], w_dst[nn12]], -1).astype(np.float32)
    si = sample_idx.astype(np.int64)

    def dlt(p, q, w=None):
        u, v = p[..., 0], p[..., 1]
        x, y = q[..., 0], q[..., 1]
        z = np.zeros_like(u)
        o = np.ones_like(u)
        r1 = np.stack([-u, -v, -o, z, z, z, x * u, x * v, x], -1)
        r2 = np.stack([z, z, z, -u, -v, -o, y * u, y * v, y], -1)
        A = np.concatenate([r1, r2], -2)
        if w is not None:
            A = A * np.concatenate([w, w], -1)[..., None]
        M = np.einsum('...ki,...kj->...ij', A, A)
        _, vecs = np.linalg.eigh(M)
        h = vecs[..., :, 0]
        H = h.reshape(h.shape[:-1] + (3, 3))
        return H / (H[..., 2:3, 2:3] + EPS)

    Hs = dlt(kp_src[si], kp_dst[si])
    sample_ok = mutual[si].all(1)
    p_hom = np.concatenate([kp_src, np.ones((N_SRC, 1), np.float32)], 1)
    counts = np.empty(MAX_ITER, np.int64)
    for i in range(0, MAX_ITER, CHUNK):
        proj = np.einsum('cij,nj->cni', Hs[i:i + CHUNK], p_hom)
        pr = proj[..., :2] / (proj[..., 2:3] + EPS)
        err = ((pr - kp_dst) ** 2).sum(-1)
        counts[i:i + CHUNK] = ((err < INL_TH) & mutual).sum(-1)
    counts = np.where(sample_ok, counts, -1)
    H_best = Hs[np.argmax(counts)].astype(np.float32)
    proj = p_hom @ H_best.T
    pr = proj[:, :2] / (proj[:, 2:3] + EPS)
    err = ((pr - kp_dst) ** 2).sum(-1)
    inl = ((err < INL_TH) & mutual).astype(np.float32)
    u, v = kp_src[:, 0], kp_src[:, 1]
    x, y = kp_dst[:, 0], kp_dst[:, 1]
    z = np.zeros_like(u)
    o = np.ones_like(u)
    r1 = np.stack([-u, -v, -o, z, z, z, x * u, x * v, x], -1)
    r2 = np.stack([z, z, z, -u, -v, -o, y * u, y * v, y], -1)
    A = np.concatenate([r1, r2], 0) * np.concatenate([inl, inl], 0)[:, None]
    _, vecs = np.linalg.eigh(A.T @ A)
    H_final = vecs[:, 0].reshape(3, 3)
    H_final = (H_final / (H_final[2, 2] + EPS)).astype(np.float32)
    ys = np.linspace(-1.0, 1.0, H_IMG, dtype=np.float32)
    xs = np.linspace(-1.0, 1.0, W_IMG, dtype=np.float32)
    gy, gx = np.meshgrid(ys, xs, indexing='ij')
    grid = np.stack([gx, gy, np.ones_like(gx)], -1)
    tg = grid @ H_final.T
    gx2 = tg[..., 0] / (tg[..., 2] + EPS)
    gy2 = tg[..., 1] / (tg[..., 2] + EPS)
    xq = (gx2 + 1.0) * 0.5 * (W_IMG - 1)
    yq = (gy2 + 1.0) * 0.5 * (H_IMG - 1)
    x0 = np.floor(xq)
    y0 = np.floor(yq)
    wx1 = xq - x0
    wy1 = yq - y0
    wx0 = 1.0 - wx1
    wy0 = 1.0 - wy1

    def gat(yi, xi):
        inb = (xi >= 0) & (xi <= W_IMG - 1) & (yi >= 0) & (yi <= H_IMG - 1)
        xc = np.clip(xi, 0, W_IMG - 1).astype(np.int32)
        yc = np.clip(yi, 0, H_IMG - 1).astype(np.int32)
        return I_src[:, yc, xc] * inb.astype(np.float32)

    out = (gat(y0, x0) * (wy0 * wx0) + gat(y0, x0 + 1) * (wy0 * wx1)
           + gat(y0 + 1, x0) * (wy1 * wx0) + gat(y0 + 1, x0 + 1) * (wy1 * wx1))
    return out[None]


def _tail(I_src, h_src, w_src, h_dst, w_dst, sample_idx, nn12, nn21):
    """Everything downstream of the device NN phase, executed with jax on
    its CPU backend so the arithmetic (eigh in particular — the polish
    homography is eigenvector-condition ~100 and the warp has an in-image
    projective pole) matches the reference bit-for-bit."""
    try:
        import jax
        import jax.numpy as jnp
        cpu = jax.devices("cpu")[0]
    except Exception:
        return _tail_np(I_src, h_src, w_src, h_dst, w_dst, sample_idx,
                        nn12, nn21)
    with jax.default_device(cpu):
        h_src = jnp.asarray(h_src)
        w_src = jnp.asarray(w_src)
        h_dst = jnp.asarray(h_dst)
        w_dst = jnp.asarray(w_dst)
        sample_idx = jnp.asarray(sample_idx)
        nn12_j = jnp.asarray(nn12)
        mutual = jnp.asarray(nn21)[nn12_j] == jnp.arange(N_SRC)

        def _dlt(p, q, w=None):
            u, v = p[:, 0], p[:, 1]
            x, y = q[:, 0], q[:, 1]
            z = jnp.zeros_like(u)
            o = jnp.ones_like(u)
            r1 = jnp.stack([-u, -v, -o, z, z, z, x * u, x * v, x], -1)
            r2 = jnp.stack([z, z, z, -u, -v, -o, y * u, y * v, y], -1)
            A = jnp.concatenate([r1, r2], 0)
            if w is not None:
                A = A * jnp.concatenate([w, w], 0)[:, None]
            _, vecs = jnp.linalg.eigh(A.T @ A)
            h = vecs[:, 0]
            H = h.reshape(3, 3)
            return H / (H[2, 2] + EPS)

        kp_src = jnp.stack([h_src, w_src], -1)
        kp_dst = jnp.stack([h_dst[nn12_j], w_dst[nn12_j]], -1)

        Hs = jax.vmap(_dlt)(kp_src[sample_idx], kp_dst[sample_idx])
        sample_ok = jnp.all(mutual[sample_idx], axis=1)

        p_hom = jnp.concatenate([kp_src, jnp.ones((N_SRC, 1),
                                                  kp_src.dtype)], 1)

        def _count(Hc):
            proj = jnp.einsum('cij,nj->cni', Hc, p_hom)
            pr = proj[..., :2] / (proj[..., 2:3] + EPS)
            err = jnp.sum((pr - kp_dst) ** 2, -1)
            return jnp.sum((err < INL_TH) & mutual, -1)

        def body(_, Hc):
            return None, _count(Hc)

        _, counts = jax.lax.scan(
            body, None, Hs.reshape(MAX_ITER // CHUNK, CHUNK, 3, 3))
        counts = jnp.where(sample_ok, counts.reshape(-1), -1)
        H_best = Hs[jnp.argmax(counts)]

        proj = p_hom @ H_best.T
        pr = proj[:, :2] / (proj[:, 2:3] + EPS)
        err = jnp.sum((pr - kp_dst) ** 2, -1)
        inl = ((err < INL_TH) & mutual).astype(kp_src.dtype)
        H_final = _dlt(kp_src, kp_dst, inl)

        ys = jnp.linspace(-1.0, 1.0, H_IMG)
        xs = jnp.linspace(-1.0, 1.0, W_IMG)
        gy, gx = jnp.meshgrid(ys, xs, indexing='ij')
        grid = jnp.stack([gx, gy, jnp.ones_like(gx)], -1)
        tg = grid @ H_final.T
        gx2 = tg[..., 0] / (tg[..., 2] + EPS)
        gy2 = tg[..., 1] / (tg[..., 2] + EPS)

        img = jnp.asarray(I_src)
        C, H, W = img.shape
        x = (gx2 + 1.0) * 0.5 * (W - 1)
        y = (gy2 + 1.0) * 0.5 * (H - 1)
        x0 = jnp.floor(x)
        y0 = jnp.floor(y)
        x1 = x0 + 1.0
        y1 = y0 + 1.0
        wx1 = x - x0
        wy1 = y - y0
        wx0 = 1.0 - wx1
        wy0 = 1.0 - wy1

        def gather(yi, xi):
            inb = (xi >= 0) & (xi <= W - 1) & (yi >= 0) & (yi <= H - 1)
            xc = jnp.clip(xi, 0, W - 1).astype(jnp.int32)
            yc = jnp.clip(yi, 0, H - 1).astype(jnp.int32)
            return img[:, yc, xc] * inb.astype(img.dtype)

        out = (gather(y0, x0) * (wy0 * wx0) + gather(y0, x1) * (wy0 * wx1)
               + gather(y1, x0) * (wy1 * wx0) + gather(y1, x1) * (wy1 * wx1))
        return np.asarray(out)[None]


def kernel(I_src, feature_src, feature_dst, h_src, w_src, h_dst, w_dst,
           sample_idx):
    I_src = np.asarray(I_src, np.float32)
    feature_src = np.asarray(feature_src, np.float32)
    feature_dst = np.asarray(feature_dst, np.float32)
    h_src = np.asarray(h_src, np.float32)
    w_src = np.asarray(w_src, np.float32)
    h_dst = np.asarray(h_dst, np.float32)
    w_dst = np.asarray(w_dst, np.float32)
    sample_idx = np.asarray(sample_idx, np.int32)

    nn12, nn21 = _run_phase1(feature_src, feature_dst)

    out = _tail(I_src, h_src, w_src, h_dst, w_dst, sample_idx, nn12, nn21)
    return out.astype(np.float32)


# revision 18
# speedup vs baseline: 1.0198x; 1.0198x over previous
"""CoarseAlignment kernel for 8 Trainium2 NeuronCores.

Device (SPMD, 8 cores): similarity matrix at fp32 accuracy via a bf16
hi/lo x3 matmul decomposition (hi@hi + hi@lo + lo@hi, 3 cyc/row vs
native fp32's 4), post-scaled by on-device feature reciprocal norms;
row argmax per src shard and col argmax via PE transposes of the
resident scaled tiles (no second matmul pass).  The fp32-level
accuracy is required: the min top-2 argmax gap is ~9e-7 and argmax
flips cascade through RANSAC into a completely different output.
Host: gather/unshard + the tiny data-dependent tail (RANSAC DLT/eigh
over 10000 hypotheses of which ~11 are sample_ok, polish, 480x640
warp) on jax's CPU backend so its near-degenerate final eigenvector
(lambda_min/lambda_2 = 0.287/0.377) and in-image projective pole match
the reference's arithmetic exactly.
"""
import os
import sys
import numpy as np

for _p in ("/opt/trn_rl_repo", "/root/.axon_site/_ro/trn_rl_repo"):
    if os.path.isdir(_p) and _p not in sys.path:
        sys.path.insert(0, _p)

import concourse.bass as bass
import concourse.mybir as mybir
from concourse import bacc
from concourse.tile import TileContext
from concourse.bass_utils import run_bass_kernel_spmd

N_SRC, N_DST, D = 8192, 2048, 1024
H_IMG, W_IMG = 480, 640
MAX_ITER = 10000
INL_TH = np.float32(0.05)
EPS = np.float32(1e-8)
CHUNK = 100
NCORES = 8
MSH = N_SRC // NCORES            # 1024 src rows per core
NKT = D // 128                   # 8 contraction tiles
PKW = MSH + N_DST                # 3072 packed columns per k-tile

_cached = {}


def _build_phase1():
    nc = bacc.Bacc(num_devices=NCORES)
    pk = nc.declare_dram_parameter("pk", [NKT, 128, 2 * PKW],
                                   mybir.dt.bfloat16, isOutput=False)
    nn12_o = nc.declare_dram_parameter("nn12", [MSH // 128, 128],
                                       mybir.dt.uint32, isOutput=True)
    colval_o = nc.declare_dram_parameter("colval", [N_DST // 128, 128],
                                         mybir.dt.float32, isOutput=True)
    colidx_o = nc.declare_dram_parameter("colidx", [N_DST // 128, 128],
                                         mybir.dt.uint32, isOutput=True)
    f32 = mybir.dt.float32
    NMT, NDT = MSH // 128, N_DST // 128
    from concourse.masks import make_identity
    with TileContext(nc) as tc:
        with tc.tile_pool(name="pks", bufs=NKT) as ppk, \
             tc.tile_pool(name="cst", bufs=1) as cst:
            P = []
            for kt in range(NKT):
                t = ppk.tile([128, 2 * PKW], mybir.dt.bfloat16, tag="pk",
                             name=f"p{kt}")
                nc.sync.dma_start(t[:], pk.ap()[kt])
                P.append(t)

            ones = cst.tile([128, 1], f32, name="ones")
            nc.gpsimd.memset(ones[:], 1.0)
            eps_t = cst.tile([1, 1], f32, name="eps_t")
            nc.gpsimd.memset(eps_t[:], float(EPS))
            ident = cst.tile([128, 128], f32, name="ident")
            make_identity(nc, ident)
            bc_dst = cst.tile([128, N_DST], f32, name="bc_dst")
            rst = cst.tile([128, MSH // 128], f32, name="rst")

            # reciprocal norms via ACT Square + ones-matmul (sum over the
            # contraction partition dim), one PSUM accumulation group
            with tc.tile_pool(name="sqp", bufs=4) as sqp, \
                 tc.tile_pool(name="nrm", bufs=1) as nrmp, \
                 tc.tile_pool(name="psn", bufs=1, space="PSUM") as psn:
                pnorm = psn.tile([1, PKW], f32, name="pnorm")
                for kt in range(NKT):
                    for j in range(PKW // 1536):
                        s = sqp.tile([128, 1536], f32, tag="s", name="s")
                        nc.vector.tensor_add(
                            s[:], P[kt][:, j * 1536:(j + 1) * 1536],
                            P[kt][:, PKW + j * 1536:PKW + (j + 1) * 1536])
                        sq = sqp.tile([128, 1536], f32, tag="sq", name="sq")
                        nc.scalar.activation(
                            sq[:], s[:], mybir.ActivationFunctionType.Square)
                        for i in range(3):
                            c0 = j * 1536 + i * 512
                            nc.tensor.matmul(
                                pnorm[:, c0:c0 + 512], ones[:],
                                sq[:, i * 512:(i + 1) * 512],
                                start=(kt == 0), stop=(kt == NKT - 1))
                rec = nrmp.tile([1, PKW], f32, name="rec")
                nc.scalar.sqrt(rec[:], pnorm[:])
                nc.vector.tensor_scalar_add(rec[:], rec[:], eps_t[:])
                nc.vector.reciprocal(rec[:], rec[:])
                nc.gpsimd.partition_broadcast(bc_dst[:], rec[:, MSH:PKW])
                rs_dram = nc.dram_tensor("rs_dram", [MSH], f32)
                nc.sync.dma_start(rs_dram.ap(), rec[:, 0:MSH])
                nc.sync.dma_start(
                    rst[:], rs_dram.ap().rearrange("(mt p) -> p mt", p=128))

            # row pass: fp32 sim, scaled by 1/dst-norm; all 8 scaled tiles
            # stay resident for the transpose-based col argmax
            with tc.tile_pool(name="psr", bufs=3, space="PSUM") as psr, \
                 tc.tile_pool(name="pst", bufs=2, space="PSUM") as pst, \
                 tc.tile_pool(name="scl", bufs=NMT) as sclp, \
                 tc.tile_pool(name="mx", bufs=2) as mxp:
                scaled_tiles = []
                for mt in range(NMT):
                    scaled = sclp.tile([128, N_DST], f32, tag="scaled",
                                       name="scaled")
                    scaled_tiles.append(scaled)
                    for half in range(2):
                        ps = psr.tile([128, 1024], f32, tag="psr", name="ps")
                        for nt in range(2):
                            col0 = half * 1024 + nt * 512
                            for kt in range(NKT):
                                hm = P[kt][:, mt * 128:(mt + 1) * 128]
                                lm = P[kt][:, PKW + mt * 128:
                                           PKW + (mt + 1) * 128]
                                hd = P[kt][:, MSH + col0:MSH + col0 + 512]
                                ld = P[kt][:, PKW + MSH + col0:
                                           PKW + MSH + col0 + 512]
                                nc.tensor.matmul(
                                    ps[:, nt * 512:(nt + 1) * 512], hm, hd,
                                    start=(kt == 0), stop=False)
                                nc.tensor.matmul(
                                    ps[:, nt * 512:(nt + 1) * 512], hm, ld,
                                    start=False, stop=False)
                                nc.tensor.matmul(
                                    ps[:, nt * 512:(nt + 1) * 512], lm, hd,
                                    start=False, stop=(kt == NKT - 1))
                        nc.vector.tensor_mul(
                            scaled[:, half * 1024:(half + 1) * 1024],
                            ps[:], bc_dst[:, half * 1024:(half + 1) * 1024])
                    nc.vector.tensor_scalar_mul(scaled[:], scaled[:],
                                                rst[:, mt:mt + 1])
                    mx = mxp.tile([128, 8], f32, tag="mx", name="mx")
                    idx = mxp.tile([128, 8], mybir.dt.uint32, tag="idx",
                                   name="idx")
                    nc.vector.max(mx[:], scaled[:])
                    nc.vector.max_index(idx[:], mx[:], scaled[:])
                    nc.sync.dma_start(nn12_o.ap()[mt], idx[:, 0:1])

                # col argmax: dt-major PE transposes so early dt tiles
                # finish early and the DVE tail pipelines
                with tc.tile_pool(name="sclc", bufs=2) as sclcp, \
                     tc.tile_pool(name="mxc", bufs=2) as mxcp:
                    for dt in range(NDT):
                        simT = sclcp.tile([128, MSH], f32, tag="simT",
                                          name="simT")
                        for mt in range(NMT):
                            pt = pst.tile([128, 128], f32, tag="pt",
                                          name="pt")
                            nc.tensor.transpose(
                                pt[:],
                                scaled_tiles[mt][:, dt * 128:(dt + 1) * 128],
                                ident[:])
                            nc.scalar.copy(
                                simT[:, mt * 128:(mt + 1) * 128], pt[:])
                        mx = mxcp.tile([128, 8], f32, tag="mxc", name="mxc")
                        idx = mxcp.tile([128, 8], mybir.dt.uint32, tag="idxc",
                                        name="idxc")
                        nc.vector.max(mx[:], simT[:])
                        nc.vector.max_index(idx[:], mx[:], simT[:])
                        nc.sync.dma_start(colval_o.ap()[dt], mx[:, 0:1])
                        nc.sync.dma_start(colidx_o.ap()[dt], idx[:, 0:1])
    nc.compile()
    return nc


def _run_phase1(feature_src, feature_dst):
    if "nc1" not in _cached:
        _cached["nc1"] = _build_phase1()
    nc = _cached["nc1"]
    import ml_dtypes
    bf16 = ml_dtypes.bfloat16
    srcT = np.ascontiguousarray(feature_src.T)          # [D, N_SRC]
    dstT = np.ascontiguousarray(feature_dst.T)          # [D, N_DST]
    in_maps = []
    dst_hi = dstT.astype(bf16)
    dst_lo = (dstT - dst_hi.astype(np.float32)).astype(bf16)
    for c in range(NCORES):
        blk = srcT[:, c * MSH:(c + 1) * MSH]
        src_hi = blk.astype(bf16)
        src_lo = (blk - src_hi.astype(np.float32)).astype(bf16)
        hi = np.concatenate([src_hi.reshape(NKT, 128, MSH),
                             dst_hi.reshape(NKT, 128, N_DST)], axis=2)
        lo = np.concatenate([src_lo.reshape(NKT, 128, MSH),
                             dst_lo.reshape(NKT, 128, N_DST)], axis=2)
        pk = np.concatenate([hi, lo], axis=2)           # [NKT,128,2*PKW] bf16
        in_maps.append({"pk": np.ascontiguousarray(pk)})
    res = run_bass_kernel_spmd(nc, in_maps, list(range(NCORES)))
    nn12 = np.concatenate([res.results[c]["nn12"].reshape(-1)
                           for c in range(NCORES)]).astype(np.int64)
    vals = np.stack([res.results[c]["colval"].reshape(-1)
                     for c in range(NCORES)])            # [8, N_DST]
    idxs = np.stack([res.results[c]["colidx"].reshape(-1)
                     for c in range(NCORES)])            # [8, N_DST]
    cbest = np.argmax(vals, axis=0)                      # first-max tiebreak
    nn21 = (idxs[cbest, np.arange(N_DST)] + cbest * MSH).astype(np.int64)
    return nn12, nn21


def _tail_np(I_src, h_src, w_src, h_dst, w_dst, sample_idx, nn12, nn21):
    # numpy fallback (used only if no jax CPU backend is importable)
    mutual = nn21[nn12] == np.arange(N_SRC)
    kp_src = np.stack([h_src, w_src], -1)
    kp_dst = np.stack([h_dst[nn12], w_dst[nn12]], -1).astype(np.float32)
    si = sample_idx.astype(np.int64)

    def dlt(p, q, w=None):
        u, v = p[..., 0], p[..., 1]
        x, y = q[..., 0], q[..., 1]
        z = np.zeros_like(u)
        o = np.ones_like(u)
        r1 = np.stack([-u, -v, -o, z, z, z, x * u, x * v, x], -1)
        r2 = np.stack([z, z, z, -u, -v, -o, y * u, y * v, y], -1)
        A = np.concatenate([r1, r2], -2)
        if w is not None:
            A = A * np.concatenate([w, w], -1)[..., None]
        M = np.einsum('...ki,...kj->...ij', A, A)
        _, vecs = np.linalg.eigh(M)
        h = vecs[..., :, 0]
        H = h.reshape(h.shape[:-1] + (3, 3))
        return H / (H[..., 2:3, 2:3] + EPS)

    Hs = dlt(kp_src[si], kp_dst[si])
    sample_ok = mutual[si].all(1)
    p_hom = np.concatenate([kp_src, np.ones((N_SRC, 1), np.float32)], 1)
    counts = np.empty(MAX_ITER, np.int64)
    for i in range(0, MAX_ITER, CHUNK):
        proj = np.einsum('cij,nj->cni', Hs[i:i + CHUNK], p_hom)
        pr = proj[..., :2] / (proj[..., 2:3] + EPS)
        err = ((pr - kp_dst) ** 2).sum(-1)
        counts[i:i + CHUNK] = ((err < INL_TH) & mutual).sum(-1)
    counts = np.where(sample_ok, counts, -1)
    H_best = Hs[np.argmax(counts)].astype(np.float32)
    proj = p_hom @ H_best.T
    pr = proj[:, :2] / (proj[:, 2:3] + EPS)
    err = ((pr - kp_dst) ** 2).sum(-1)
    inl = ((err < INL_TH) & mutual).astype(np.float32)
    u, v = kp_src[:, 0], kp_src[:, 1]
    x, y = kp_dst[:, 0], kp_dst[:, 1]
    z = np.zeros_like(u)
    o = np.ones_like(u)
    r1 = np.stack([-u, -v, -o, z, z, z, x * u, x * v, x], -1)
    r2 = np.stack([z, z, z, -u, -v, -o, y * u, y * v, y], -1)
    A = np.concatenate([r1, r2], 0) * np.concatenate([inl, inl], 0)[:, None]
    _, vecs = np.linalg.eigh(A.T @ A)
    H_final = vecs[:, 0].reshape(3, 3)
    H_final = (H_final / (H_final[2, 2] + EPS)).astype(np.float32)
    ys = np.linspace(-1.0, 1.0, H_IMG, dtype=np.float32)
    xs = np.linspace(-1.0, 1.0, W_IMG, dtype=np.float32)
    gy, gx = np.meshgrid(ys, xs, indexing='ij')
    grid = np.stack([gx, gy, np.ones_like(gx)], -1)
    tg = grid @ H_final.T
    gx2 = tg[..., 0] / (tg[..., 2] + EPS)
    gy2 = tg[..., 1] / (tg[..., 2] + EPS)
    xq = (gx2 + 1.0) * 0.5 * (W_IMG - 1)
    yq = (gy2 + 1.0) * 0.5 * (H_IMG - 1)
    x0 = np.floor(xq)
    y0 = np.floor(yq)
    wx1 = xq - x0
    wy1 = yq - y0
    wx0 = 1.0 - wx1
    wy0 = 1.0 - wy1

    def gat(yi, xi):
        inb = (xi >= 0) & (xi <= W_IMG - 1) & (yi >= 0) & (yi <= H_IMG - 1)
        xc = np.clip(xi, 0, W_IMG - 1).astype(np.int32)
        yc = np.clip(yi, 0, H_IMG - 1).astype(np.int32)
        return I_src[:, yc, xc] * inb.astype(np.float32)

    out = (gat(y0, x0) * (wy0 * wx0) + gat(y0, x0 + 1) * (wy0 * wx1)
           + gat(y0 + 1, x0) * (wy1 * wx0) + gat(y0 + 1, x0 + 1) * (wy1 * wx1))
    return out[None]


def _tail(I_src, h_src, w_src, h_dst, w_dst, sample_idx, nn12, nn21):
    """Everything downstream of the device NN phase, executed with jax on
    its CPU backend so the arithmetic (eigh in particular — the polish
    homography is eigenvector-condition ~100 and the warp has an in-image
    projective pole) matches the reference bit-for-bit."""
    try:
        import jax
        import jax.numpy as jnp
        cpu = jax.devices("cpu")[0]
    except Exception:
        return _tail_np(I_src, h_src, w_src, h_dst, w_dst, sample_idx,
                        nn12, nn21)
    with jax.default_device(cpu):
        h_src = jnp.asarray(h_src)
        w_src = jnp.asarray(w_src)
        h_dst = jnp.asarray(h_dst)
        w_dst = jnp.asarray(w_dst)
        sample_idx = jnp.asarray(sample_idx)
        nn12_j = jnp.asarray(nn12)
        mutual = jnp.asarray(nn21)[nn12_j] == jnp.arange(N_SRC)

        def _dlt(p, q, w=None):
            u, v = p[:, 0], p[:, 1]
            x, y = q[:, 0], q[:, 1]
            z = jnp.zeros_like(u)
            o = jnp.ones_like(u)
            r1 = jnp.stack([-u, -v, -o, z, z, z, x * u, x * v, x], -1)
            r2 = jnp.stack([z, z, z, -u, -v, -o, y * u, y * v, y], -1)
            A = jnp.concatenate([r1, r2], 0)
            if w is not None:
                A = A * jnp.concatenate([w, w], 0)[:, None]
            _, vecs = jnp.linalg.eigh(A.T @ A)
            h = vecs[:, 0]
            H = h.reshape(3, 3)
            return H / (H[2, 2] + EPS)

        kp_src = jnp.stack([h_src, w_src], -1)
        kp_dst = jnp.stack([h_dst[nn12_j], w_dst[nn12_j]], -1)

        Hs = jax.vmap(_dlt)(kp_src[sample_idx], kp_dst[sample_idx])
        sample_ok = jnp.all(mutual[sample_idx], axis=1)

        p_hom = jnp.concatenate([kp_src, jnp.ones((N_SRC, 1),
                                                  kp_src.dtype)], 1)

        def _count(Hc):
            proj = jnp.einsum('cij,nj->cni', Hc, p_hom)
            pr = proj[..., :2] / (proj[..., 2:3] + EPS)
            err = jnp.sum((pr - kp_dst) ** 2, -1)
            return jnp.sum((err < INL_TH) & mutual, -1)

        def body(_, Hc):
            return None, _count(Hc)

        _, counts = jax.lax.scan(
            body, None, Hs.reshape(MAX_ITER // CHUNK, CHUNK, 3, 3))
        counts = jnp.where(sample_ok, counts.reshape(-1), -1)
        H_best = Hs[jnp.argmax(counts)]

        proj = p_hom @ H_best.T
        pr = proj[:, :2] / (proj[:, 2:3] + EPS)
        err = jnp.sum((pr - kp_dst) ** 2, -1)
        inl = ((err < INL_TH) & mutual).astype(kp_src.dtype)
        H_final = _dlt(kp_src, kp_dst, inl)

        ys = jnp.linspace(-1.0, 1.0, H_IMG)
        xs = jnp.linspace(-1.0, 1.0, W_IMG)
        gy, gx = jnp.meshgrid(ys, xs, indexing='ij')
        grid = jnp.stack([gx, gy, jnp.ones_like(gx)], -1)
        tg = grid @ H_final.T
        gx2 = tg[..., 0] / (tg[..., 2] + EPS)
        gy2 = tg[..., 1] / (tg[..., 2] + EPS)

        img = jnp.asarray(I_src)
        C, H, W = img.shape
        x = (gx2 + 1.0) * 0.5 * (W - 1)
        y = (gy2 + 1.0) * 0.5 * (H - 1)
        x0 = jnp.floor(x)
        y0 = jnp.floor(y)
        x1 = x0 + 1.0
        y1 = y0 + 1.0
        wx1 = x - x0
        wy1 = y - y0
        wx0 = 1.0 - wx1
        wy0 = 1.0 - wy1

        def gather(yi, xi):
            inb = (xi >= 0) & (xi <= W - 1) & (yi >= 0) & (yi <= H - 1)
            xc = jnp.clip(xi, 0, W - 1).astype(jnp.int32)
            yc = jnp.clip(yi, 0, H - 1).astype(jnp.int32)
            return img[:, yc, xc] * inb.astype(img.dtype)

        out = (gather(y0, x0) * (wy0 * wx0) + gather(y0, x1) * (wy0 * wx1)
               + gather(y1, x0) * (wy1 * wx0) + gather(y1, x1) * (wy1 * wx1))
        return np.asarray(out)[None]


def kernel(I_src, feature_src, feature_dst, h_src, w_src, h_dst, w_dst,
           sample_idx):
    I_src = np.asarray(I_src, np.float32)
    feature_src = np.asarray(feature_src, np.float32)
    feature_dst = np.asarray(feature_dst, np.float32)
    h_src = np.asarray(h_src, np.float32)
    w_src = np.asarray(w_src, np.float32)
    h_dst = np.asarray(h_dst, np.float32)
    w_dst = np.asarray(w_dst, np.float32)
    sample_idx = np.asarray(sample_idx, np.int32)

    nn12, nn21 = _run_phase1(feature_src, feature_dst)

    out = _tail(I_src, h_src, w_src, h_dst, w_dst, sample_idx, nn12, nn21)
    return out.astype(np.float32)


# revision 19
# speedup vs baseline: 1.0568x; 1.0363x over previous
"""CoarseAlignment kernel for 8 Trainium2 NeuronCores.

Device (SPMD, 8 cores): similarity matrix at fp32 accuracy via a bf16
hi/lo x3 matmul decomposition (hi@hi + hi@lo + lo@hi, 3 cyc/row vs
native fp32's 4), post-scaled by on-device feature reciprocal norms;
row argmax per src shard and col argmax via PE transposes of the
resident scaled tiles (no second matmul pass).  The fp32-level
accuracy is required: the min top-2 argmax gap is ~9e-7 and argmax
flips cascade through RANSAC into a completely different output.
Host: gather/unshard + the tiny data-dependent tail (RANSAC DLT/eigh
over 10000 hypotheses of which ~11 are sample_ok, polish, 480x640
warp) on jax's CPU backend so its near-degenerate final eigenvector
(lambda_min/lambda_2 = 0.287/0.377) and in-image projective pole match
the reference's arithmetic exactly.
"""
import os
import sys
import numpy as np

for _p in ("/opt/trn_rl_repo", "/root/.axon_site/_ro/trn_rl_repo"):
    if os.path.isdir(_p) and _p not in sys.path:
        sys.path.insert(0, _p)

import concourse.bass as bass
import concourse.mybir as mybir
from concourse import bacc
from concourse.tile import TileContext
from concourse.bass_utils import run_bass_kernel_spmd

N_SRC, N_DST, D = 8192, 2048, 1024
H_IMG, W_IMG = 480, 640
MAX_ITER = 10000
INL_TH = np.float32(0.05)
EPS = np.float32(1e-8)
CHUNK = 100
NCORES = 8
MSH = N_SRC // NCORES            # 1024 src rows per core
NKT = D // 128                   # 8 contraction tiles
PKW = MSH + N_DST                # 3072 packed columns per k-tile

_cached = {}


def _build_phase1():
    nc = bacc.Bacc(num_devices=NCORES)
    pk = nc.declare_dram_parameter("pk", [NKT, 128, 2 * PKW],
                                   mybir.dt.bfloat16, isOutput=False)
    nn12_o = nc.declare_dram_parameter("nn12", [MSH // 128, 128],
                                       mybir.dt.uint32, isOutput=True)
    colval_o = nc.declare_dram_parameter("colval", [N_DST // 128, 128],
                                         mybir.dt.float32, isOutput=True)
    colidx_o = nc.declare_dram_parameter("colidx", [N_DST // 128, 128],
                                         mybir.dt.uint32, isOutput=True)
    f32 = mybir.dt.float32
    NMT, NDT = MSH // 128, N_DST // 128
    from concourse.masks import make_identity
    with TileContext(nc) as tc:
        with tc.tile_pool(name="pks", bufs=NKT) as ppk, \
             tc.tile_pool(name="cst", bufs=1) as cst:
            P = []
            for kt in range(NKT):
                t = ppk.tile([128, 2 * PKW], mybir.dt.bfloat16, tag="pk",
                             name=f"p{kt}")
                nc.sync.dma_start(t[:], pk.ap()[kt])
                P.append(t)

            ones = cst.tile([128, 1], f32, name="ones")
            nc.gpsimd.memset(ones[:], 1.0)
            eps_t = cst.tile([1, 1], f32, name="eps_t")
            nc.gpsimd.memset(eps_t[:], float(EPS))
            ident = cst.tile([128, 128], f32, name="ident")
            make_identity(nc, ident)
            bc_dst = cst.tile([128, N_DST], f32, name="bc_dst")
            rst = cst.tile([128, MSH // 128], f32, name="rst")

            # reciprocal norms via ACT Square + ones-matmul (sum over the
            # contraction partition dim), one PSUM accumulation group
            with tc.tile_pool(name="sqp", bufs=4) as sqp, \
                 tc.tile_pool(name="nrm", bufs=1) as nrmp, \
                 tc.tile_pool(name="psn", bufs=1, space="PSUM") as psn:
                pnorm = psn.tile([1, PKW], f32, name="pnorm")
                for kt in range(NKT):
                    for j in range(PKW // 1536):
                        s = sqp.tile([128, 1536], f32, tag="s", name="s")
                        nc.vector.tensor_add(
                            s[:], P[kt][:, j * 1536:(j + 1) * 1536],
                            P[kt][:, PKW + j * 1536:PKW + (j + 1) * 1536])
                        sq = sqp.tile([128, 1536], f32, tag="sq", name="sq")
                        nc.scalar.activation(
                            sq[:], s[:], mybir.ActivationFunctionType.Square)
                        for i in range(3):
                            c0 = j * 1536 + i * 512
                            nc.tensor.matmul(
                                pnorm[:, c0:c0 + 512], ones[:],
                                sq[:, i * 512:(i + 1) * 512],
                                start=(kt == 0), stop=(kt == NKT - 1))
                rec = nrmp.tile([1, PKW], f32, name="rec")
                nc.scalar.sqrt(rec[:], pnorm[:])
                nc.vector.tensor_scalar_add(rec[:], rec[:], eps_t[:])
                nc.vector.reciprocal(rec[:], rec[:])
                nc.gpsimd.partition_broadcast(bc_dst[:], rec[:, MSH:PKW])
                rs_dram = nc.dram_tensor("rs_dram", [MSH], f32)
                nc.sync.dma_start(rs_dram.ap(), rec[:, 0:MSH])
                nc.sync.dma_start(
                    rst[:], rs_dram.ap().rearrange("(mt p) -> p mt", p=128))

            # row pass: fp32 sim, scaled by 1/dst-norm; all 8 scaled tiles
            # stay resident for the transpose-based col argmax
            with tc.tile_pool(name="psr", bufs=2, space="PSUM") as psr, \
                 tc.tile_pool(name="pst", bufs=2, space="PSUM") as pst, \
                 tc.tile_pool(name="scl", bufs=NMT) as sclp, \
                 tc.tile_pool(name="mx", bufs=2) as mxp:
                scaled_tiles = []
                for mt in range(NMT):
                    scaled = sclp.tile([128, N_DST], f32, tag="scaled",
                                       name="scaled")
                    scaled_tiles.append(scaled)
                    for half in range(2):
                        ps = psr.tile([128, 1024], f32, tag="psr", name="ps")
                        for nt in range(2):
                            col0 = half * 1024 + nt * 512
                            for kt in range(NKT):
                                hm = P[kt][:, mt * 128:(mt + 1) * 128]
                                lm = P[kt][:, PKW + mt * 128:
                                           PKW + (mt + 1) * 128]
                                hd = P[kt][:, MSH + col0:MSH + col0 + 512]
                                ld = P[kt][:, PKW + MSH + col0:
                                           PKW + MSH + col0 + 512]
                                nc.tensor.matmul(
                                    ps[:, nt * 512:(nt + 1) * 512], hm, hd,
                                    start=(kt == 0), stop=False)
                                nc.tensor.matmul(
                                    ps[:, nt * 512:(nt + 1) * 512], hm, ld,
                                    start=False, stop=False)
                                nc.tensor.matmul(
                                    ps[:, nt * 512:(nt + 1) * 512], lm, hd,
                                    start=False, stop=(kt == NKT - 1))
                        nc.vector.tensor_mul(
                            scaled[:, half * 1024:(half + 1) * 1024],
                            ps[:], bc_dst[:, half * 1024:(half + 1) * 1024])
                    nc.vector.tensor_scalar_mul(scaled[:], scaled[:],
                                                rst[:, mt:mt + 1])
                    mx = mxp.tile([128, 8], f32, tag="mx", name="mx")
                    idx = mxp.tile([128, 8], mybir.dt.uint32, tag="idx",
                                   name="idx")
                    nc.vector.max(mx[:], scaled[:])
                    nc.vector.max_index(idx[:], mx[:], scaled[:])
                    nc.sync.dma_start(nn12_o.ap()[mt], idx[:, 0:1])

                # col argmax: dt-major PE transposes so early dt tiles
                # finish early and the DVE tail pipelines
                with tc.tile_pool(name="sclc", bufs=2) as sclcp, \
                     tc.tile_pool(name="mxc", bufs=2) as mxcp:
                    for dt in range(NDT):
                        simT = sclcp.tile([128, MSH], f32, tag="simT",
                                          name="simT")
                        ptT = pst.tile([128, MSH], f32, tag="ptT",
                                       name="ptT")
                        for mt in range(NMT):
                            nc.tensor.transpose(
                                ptT[:, mt * 128:(mt + 1) * 128],
                                scaled_tiles[mt][:, dt * 128:(dt + 1) * 128],
                                ident[:])
                        nc.scalar.copy(simT[:], ptT[:])
                        mx = mxcp.tile([128, 8], f32, tag="mxc", name="mxc")
                        idx = mxcp.tile([128, 8], mybir.dt.uint32, tag="idxc",
                                        name="idxc")
                        nc.vector.max(mx[:], simT[:])
                        nc.vector.max_index(idx[:], mx[:], simT[:])
                        nc.sync.dma_start(colval_o.ap()[dt], mx[:, 0:1])
                        nc.sync.dma_start(colidx_o.ap()[dt], idx[:, 0:1])
    nc.compile()
    return nc


def _run_phase1(feature_src, feature_dst):
    if "nc1" not in _cached:
        _cached["nc1"] = _build_phase1()
    nc = _cached["nc1"]
    import ml_dtypes
    bf16 = ml_dtypes.bfloat16
    srcT = np.ascontiguousarray(feature_src.T)          # [D, N_SRC]
    dstT = np.ascontiguousarray(feature_dst.T)          # [D, N_DST]
    in_maps = []
    dst_hi = dstT.astype(bf16)
    dst_lo = (dstT - dst_hi.astype(np.float32)).astype(bf16)
    for c in range(NCORES):
        blk = srcT[:, c * MSH:(c + 1) * MSH]
        src_hi = blk.astype(bf16)
        src_lo = (blk - src_hi.astype(np.float32)).astype(bf16)
        hi = np.concatenate([src_hi.reshape(NKT, 128, MSH),
                             dst_hi.reshape(NKT, 128, N_DST)], axis=2)
        lo = np.concatenate([src_lo.reshape(NKT, 128, MSH),
                             dst_lo.reshape(NKT, 128, N_DST)], axis=2)
        pk = np.concatenate([hi, lo], axis=2)           # [NKT,128,2*PKW] bf16
        in_maps.append({"pk": np.ascontiguousarray(pk)})
    res = run_bass_kernel_spmd(nc, in_maps, list(range(NCORES)))
    nn12 = np.concatenate([res.results[c]["nn12"].reshape(-1)
                           for c in range(NCORES)]).astype(np.int64)
    vals = np.stack([res.results[c]["colval"].reshape(-1)
                     for c in range(NCORES)])            # [8, N_DST]
    idxs = np.stack([res.results[c]["colidx"].reshape(-1)
                     for c in range(NCORES)])            # [8, N_DST]
    cbest = np.argmax(vals, axis=0)                      # first-max tiebreak
    nn21 = (idxs[cbest, np.arange(N_DST)] + cbest * MSH).astype(np.int64)
    return nn12, nn21


def _tail_np(I_src, h_src, w_src, h_dst, w_dst, sample_idx, nn12, nn21):
    # numpy fallback (used only if no jax CPU backend is importable)
    mutual = nn21[nn12] == np.arange(N_SRC)
    kp_src = np.stack([h_src, w_src], -1)
    kp_dst = np.stack([h_dst[nn12], w_dst[nn12]], -1).astype(np.float32)
    si = sample_idx.astype(np.int64)

    def dlt(p, q, w=None):
        u, v = p[..., 0], p[..., 1]
        x, y = q[..., 0], q[..., 1]
        z = np.zeros_like(u)
        o = np.ones_like(u)
        r1 = np.stack([-u, -v, -o, z, z, z, x * u, x * v, x], -1)
        r2 = np.stack([z, z, z, -u, -v, -o, y * u, y * v, y], -1)
        A = np.concatenate([r1, r2], -2)
        if w is not None:
            A = A * np.concatenate([w, w], -1)[..., None]
        M = np.einsum('...ki,...kj->...ij', A, A)
        _, vecs = np.linalg.eigh(M)
        h = vecs[..., :, 0]
        H = h.reshape(h.shape[:-1] + (3, 3))
        return H / (H[..., 2:3, 2:3] + EPS)

    Hs = dlt(kp_src[si], kp_dst[si])
    sample_ok = mutual[si].all(1)
    p_hom = np.concatenate([kp_src, np.ones((N_SRC, 1), np.float32)], 1)
    counts = np.empty(MAX_ITER, np.int64)
    for i in range(0, MAX_ITER, CHUNK):
        proj = np.einsum('cij,nj->cni', Hs[i:i + CHUNK], p_hom)
        pr = proj[..., :2] / (proj[..., 2:3] + EPS)
        err = ((pr - kp_dst) ** 2).sum(-1)
        counts[i:i + CHUNK] = ((err < INL_TH) & mutual).sum(-1)
    counts = np.where(sample_ok, counts, -1)
    H_best = Hs[np.argmax(counts)].astype(np.float32)
    proj = p_hom @ H_best.T
    pr = proj[:, :2] / (proj[:, 2:3] + EPS)
    err = ((pr - kp_dst) ** 2).sum(-1)
    inl = ((err < INL_TH) & mutual).astype(np.float32)
    u, v = kp_src[:, 0], kp_src[:, 1]
    x, y = kp_dst[:, 0], kp_dst[:, 1]
    z = np.zeros_like(u)
    o = np.ones_like(u)
    r1 = np.stack([-u, -v, -o, z, z, z, x * u, x * v, x], -1)
    r2 = np.stack([z, z, z, -u, -v, -o, y * u, y * v, y], -1)
    A = np.concatenate([r1, r2], 0) * np.concatenate([inl, inl], 0)[:, None]
    _, vecs = np.linalg.eigh(A.T @ A)
    H_final = vecs[:, 0].reshape(3, 3)
    H_final = (H_final / (H_final[2, 2] + EPS)).astype(np.float32)
    ys = np.linspace(-1.0, 1.0, H_IMG, dtype=np.float32)
    xs = np.linspace(-1.0, 1.0, W_IMG, dtype=np.float32)
    gy, gx = np.meshgrid(ys, xs, indexing='ij')
    grid = np.stack([gx, gy, np.ones_like(gx)], -1)
    tg = grid @ H_final.T
    gx2 = tg[..., 0] / (tg[..., 2] + EPS)
    gy2 = tg[..., 1] / (tg[..., 2] + EPS)
    xq = (gx2 + 1.0) * 0.5 * (W_IMG - 1)
    yq = (gy2 + 1.0) * 0.5 * (H_IMG - 1)
    x0 = np.floor(xq)
    y0 = np.floor(yq)
    wx1 = xq - x0
    wy1 = yq - y0
    wx0 = 1.0 - wx1
    wy0 = 1.0 - wy1

    def gat(yi, xi):
        inb = (xi >= 0) & (xi <= W_IMG - 1) & (yi >= 0) & (yi <= H_IMG - 1)
        xc = np.clip(xi, 0, W_IMG - 1).astype(np.int32)
        yc = np.clip(yi, 0, H_IMG - 1).astype(np.int32)
        return I_src[:, yc, xc] * inb.astype(np.float32)

    out = (gat(y0, x0) * (wy0 * wx0) + gat(y0, x0 + 1) * (wy0 * wx1)
           + gat(y0 + 1, x0) * (wy1 * wx0) + gat(y0 + 1, x0 + 1) * (wy1 * wx1))
    return out[None]


def _tail(I_src, h_src, w_src, h_dst, w_dst, sample_idx, nn12, nn21):
    """Everything downstream of the device NN phase, executed with jax on
    its CPU backend so the arithmetic (eigh in particular — the polish
    homography is eigenvector-condition ~100 and the warp has an in-image
    projective pole) matches the reference bit-for-bit."""
    try:
        import jax
        import jax.numpy as jnp
        cpu = jax.devices("cpu")[0]
    except Exception:
        return _tail_np(I_src, h_src, w_src, h_dst, w_dst, sample_idx,
                        nn12, nn21)
    with jax.default_device(cpu):
        h_src = jnp.asarray(h_src)
        w_src = jnp.asarray(w_src)
        h_dst = jnp.asarray(h_dst)
        w_dst = jnp.asarray(w_dst)
        sample_idx = jnp.asarray(sample_idx)
        nn12_j = jnp.asarray(nn12)
        mutual = jnp.asarray(nn21)[nn12_j] == jnp.arange(N_SRC)

        def _dlt(p, q, w=None):
            u, v = p[:, 0], p[:, 1]
            x, y = q[:, 0], q[:, 1]
            z = jnp.zeros_like(u)
            o = jnp.ones_like(u)
            r1 = jnp.stack([-u, -v, -o, z, z, z, x * u, x * v, x], -1)
            r2 = jnp.stack([z, z, z, -u, -v, -o, y * u, y * v, y], -1)
            A = jnp.concatenate([r1, r2], 0)
            if w is not None:
                A = A * jnp.concatenate([w, w], 0)[:, None]
            _, vecs = jnp.linalg.eigh(A.T @ A)
            h = vecs[:, 0]
            H = h.reshape(3, 3)
            return H / (H[2, 2] + EPS)

        kp_src = jnp.stack([h_src, w_src], -1)
        kp_dst = jnp.stack([h_dst[nn12_j], w_dst[nn12_j]], -1)

        Hs = jax.vmap(_dlt)(kp_src[sample_idx], kp_dst[sample_idx])
        sample_ok = jnp.all(mutual[sample_idx], axis=1)

        p_hom = jnp.concatenate([kp_src, jnp.ones((N_SRC, 1),
                                                  kp_src.dtype)], 1)

        def _count(Hc):
            proj = jnp.einsum('cij,nj->cni', Hc, p_hom)
            pr = proj[..., :2] / (proj[..., 2:3] + EPS)
            err = jnp.sum((pr - kp_dst) ** 2, -1)
            return jnp.sum((err < INL_TH) & mutual, -1)

        def body(_, Hc):
            return None, _count(Hc)

        _, counts = jax.lax.scan(
            body, None, Hs.reshape(MAX_ITER // CHUNK, CHUNK, 3, 3))
        counts = jnp.where(sample_ok, counts.reshape(-1), -1)
        H_best = Hs[jnp.argmax(counts)]

        proj = p_hom @ H_best.T
        pr = proj[:, :2] / (proj[:, 2:3] + EPS)
        err = jnp.sum((pr - kp_dst) ** 2, -1)
        inl = ((err < INL_TH) & mutual).astype(kp_src.dtype)
        H_final = _dlt(kp_src, kp_dst, inl)

        ys = jnp.linspace(-1.0, 1.0, H_IMG)
        xs = jnp.linspace(-1.0, 1.0, W_IMG)
        gy, gx = jnp.meshgrid(ys, xs, indexing='ij')
        grid = jnp.stack([gx, gy, jnp.ones_like(gx)], -1)
        tg = grid @ H_final.T
        gx2 = tg[..., 0] / (tg[..., 2] + EPS)
        gy2 = tg[..., 1] / (tg[..., 2] + EPS)

        img = jnp.asarray(I_src)
        C, H, W = img.shape
        x = (gx2 + 1.0) * 0.5 * (W - 1)
        y = (gy2 + 1.0) * 0.5 * (H - 1)
        x0 = jnp.floor(x)
        y0 = jnp.floor(y)
        x1 = x0 + 1.0
        y1 = y0 + 1.0
        wx1 = x - x0
        wy1 = y - y0
        wx0 = 1.0 - wx1
        wy0 = 1.0 - wy1

        def gather(yi, xi):
            inb = (xi >= 0) & (xi <= W - 1) & (yi >= 0) & (yi <= H - 1)
            xc = jnp.clip(xi, 0, W - 1).astype(jnp.int32)
            yc = jnp.clip(yi, 0, H - 1).astype(jnp.int32)
            return img[:, yc, xc] * inb.astype(img.dtype)

        out = (gather(y0, x0) * (wy0 * wx0) + gather(y0, x1) * (wy0 * wx1)
               + gather(y1, x0) * (wy1 * wx0) + gather(y1, x1) * (wy1 * wx1))
        return np.asarray(out)[None]


def kernel(I_src, feature_src, feature_dst, h_src, w_src, h_dst, w_dst,
           sample_idx):
    I_src = np.asarray(I_src, np.float32)
    feature_src = np.asarray(feature_src, np.float32)
    feature_dst = np.asarray(feature_dst, np.float32)
    h_src = np.asarray(h_src, np.float32)
    w_src = np.asarray(w_src, np.float32)
    h_dst = np.asarray(h_dst, np.float32)
    w_dst = np.asarray(w_dst, np.float32)
    sample_idx = np.asarray(sample_idx, np.int32)

    nn12, nn21 = _run_phase1(feature_src, feature_dst)

    out = _tail(I_src, h_src, w_src, h_dst, w_dst, sample_idx, nn12, nn21)
    return out.astype(np.float32)
